# revision 1
# baseline (speedup 1.0000x reference)
"""Trainium2 Bass kernel for 3D neighborhood attention (sparse_attention).

Problem: q,k [1,40,40,40,48] fp32, rpb [8,3,3,3]; out [1,24,40,40,40].
Per voxel x: logits[h,kk] = scale * <q[x,h,:], k[x+off_kk,h,:]> + rpb[h,kk]
(zero-padded k at boundaries, kk over 3x3x3 offsets), p = softmax over kk,
out[x,h,:] = sum_kk p[h,kk] * off_kk  (constant integer offsets as values).

Sharding: spatial-parallel over H (40 -> 8 slabs of 5). Each core gets its
q slab plus a host-side im2col of the 27 shifted k views for its slab
(halo handled on host), so on-core everything is token-parallel with
tokens on SBUF partitions (2 tokens per partition) and no cross-partition
data movement. The PV contraction exploits that the "values" are the
constant offsets in {-1,0,1}^3: out_i = (sum of exp over di=+1 block) -
(sum over di=-1 block), so it is pure block reductions, no multiplies.
"""

import numpy as np

import concourse.bass as bass
import concourse.tile as tile
from concourse import bacc, mybir
from concourse.bass_utils import run_bass_kernel_spmd

NH = 8
HD = 6
DIM = NH * HD
KS = 3
NT = KS**3  # 27
SCALE = HD**-0.5
H = W = T = 40
N_CORES = 8
SLAB = H // N_CORES          # 5 rows of H per core
TOK = SLAB * W * T           # 8000 tokens per core
P = 128
TPP = 2                      # tokens per partition
TILES = 32                   # ceil(8000 / 256)
TOKP = TILES * P * TPP       # 8192
FKC = NT * DIM               # 1296  (kk, c) free dim per token
FKH = NT * NH                # 216   (kk, h) free dim per token

_prog_cache = {}


def _build_program():
    fp32 = mybir.dt.float32
    nc = bacc.Bacc("TRN2", target_bir_lowering=False, debug=False,
                   num_devices=N_CORES)
    qs = nc.dram_tensor("qs", [TILES, P, TPP * DIM], fp32,
                        kind="ExternalInput").ap()
    kn = nc.dram_tensor("kn", [TILES, P, TPP * FKC], fp32,
                        kind="ExternalInput").ap()
    rpbt = nc.dram_tensor("rpbt", [P, FKH], fp32, kind="ExternalInput").ap()
    out = nc.dram_tensor("out", [TILES, P, TPP * 3 * NH], fp32,
                         kind="ExternalOutput").ap()

    X = mybir.AxisListType.X
    XY = mybir.AxisListType.XY
    ADD = mybir.AluOpType.add

    with tile.TileContext(nc) as tc:
        with (
            tc.tile_pool(name="consts", bufs=1) as cpool,
            tc.tile_pool(name="kin", bufs=3) as kpool,
            tc.tile_pool(name="qin", bufs=3) as qpool,
            tc.tile_pool(name="prod", bufs=2) as ppool,
            tc.tile_pool(name="logit", bufs=3) as lpool,
            tc.tile_pool(name="expv", bufs=3) as epool,
            tc.tile_pool(name="small", bufs=16) as spool,
            tc.tile_pool(name="outp", bufs=3) as opool,
        ):
            rpb_sb = cpool.tile([P, FKH], fp32)
            nc.sync.dma_start(rpb_sb[:], rpbt[:])

            for ti in range(TILES):
                kt = kpool.tile([P, TPP * FKC], fp32)
                nc.sync.dma_start(kt[:], kn[ti])
                qt = qpool.tile([P, TPP * DIM], fp32)
                nc.sync.dma_start(qt[:], qs[ti])

                # P4[p, j, kk, c] = kn[p, j, kk, c] * q[p, j, c]
                pt = ppool.tile([P, TPP * FKC], fp32)
                q_b = (qt[:].rearrange("p (j c) -> p j c", j=TPP)
                       .unsqueeze(2).broadcast_to([P, TPP, NT, DIM]))
                nc.vector.tensor_mul(
                    pt[:].rearrange("p (j kk c) -> p j kk c", j=TPP, kk=NT),
                    kt[:].rearrange("p (j kk c) -> p j kk c", j=TPP, kk=NT),
                    q_b,
                )
                # L[p, (j,kk,h)] = sum_d P4[p, j, (kk,h), d]
                lt = lpool.tile([P, TPP * FKH], fp32)
                nc.vector.tensor_reduce(
                    lt[:],
                    pt[:].rearrange("p (j kh d) -> p j kh d", j=TPP, d=HD),
                    axis=X, op=ADD,
                )
                # L2 = L + rpb  (q was pre-scaled by SCALE on host)
                l2 = lpool.tile([P, TPP * FKH], fp32)
                rpb_b = rpb_sb[:].unsqueeze(1).broadcast_to([P, TPP, FKH])
                nc.vector.tensor_add(
                    l2[:].rearrange("p (j f) -> p j f", j=TPP),
                    lt[:].rearrange("p (j f) -> p j f", j=TPP),
                    rpb_b,
                )
                # E = exp(L2)  (ScalarE, overlaps with DVE)
                et = epool.tile([P, TPP * FKH], fp32)
                nc.scalar.activation(et[:], l2[:],
                                     mybir.ActivationFunctionType.Exp)

                # Softmax denominator: S0[p, (j,h)] = sum_kk E
                e_khk = et[:].rearrange("p (j kk h) -> p j h kk",
                                        j=TPP, kk=NT, h=NH)
                s0 = spool.tile([P, TPP * NH], fp32)
                nc.vector.tensor_reduce(s0[:], e_khk, axis=X, op=ADD)

                # Directional numerators via paired block sums over the
                # +-1 slabs of each axis (values are +-1/0).
                # E free layout: (j, di, dj, dl, h).  V layout: (o, j, pm, h)
                v_di = et[:].rearrange(
                    "p (j di dj dl h) -> p j di h (dj dl)",
                    j=TPP, di=KS, dj=KS, dl=KS, h=NH)
                v_dj = et[:].rearrange(
                    "p (j di dj dl h) -> p j dj h di dl",
                    j=TPP, di=KS, dj=KS, dl=KS, h=NH)
                v_dl = et[:].rearrange(
                    "p (j di dj dl h) -> p j dl h di dj",
                    j=TPP, di=KS, dj=KS, dl=KS, h=NH)

                vt = spool.tile([P, 3 * 2 * TPP * NH], fp32)  # [128, 96]
                npm = TPP * NH
                for o, (v, ax) in enumerate(((v_di, X), (v_dj, XY),
                                             (v_dl, XY))):
                    for pm in range(2):
                        nc.vector.tensor_reduce(
                            vt[:, (o * 2 + pm) * npm:(o * 2 + pm + 1) * npm],
                            v[:, :, 2 * pm], axis=ax, op=ADD)

                # S3[p, (o,j,h)] = V[.., pm=1] - V[.., pm=0]
                v5 = vt[:].rearrange("p (o pm j h) -> p o pm j h",
                                     o=3, pm=2, j=TPP)
                s3 = spool.tile([P, 3 * TPP * NH], fp32)
                nc.vector.tensor_sub(
                    s3[:].rearrange("p (o j h) -> p o j h", o=3, j=TPP),
                    v5[:, :, 1], v5[:, :, 0])

                rt = spool.tile([P, TPP * NH], fp32)
                nc.vector.reciprocal(rt[:], s0[:])
                # out[p, (o,j,h)] = S3 * (1/S0)
                ot = opool.tile([P, TPP * 3 * NH], fp32)
                r_b = (rt[:].rearrange("p (j h) -> p j h", j=TPP)
                       .unsqueeze(1).broadcast_to([P, 3, TPP, NH]))
                nc.vector.tensor_mul(
                    ot[:].rearrange("p (o j h) -> p o j h", o=3, j=TPP),
                    s3[:].rearrange("p (o j h) -> p o j h", o=3, j=TPP),
                    r_b)
                nc.sync.dma_start(out[ti], ot[:])

    nc.compile()
    return nc


def _host_prep(q, k, rpb):
    q = np.asarray(q, dtype=np.float32)
    k = np.asarray(k, dtype=np.float32)
    rpb = np.asarray(rpb, dtype=np.float32)

    q0 = (q[0] * SCALE).astype(np.float32)          # [40,40,40,48]
    kp = np.pad(k[0], ((1, 1), (1, 1), (1, 1), (0, 0)))  # [42,42,42,48]
    win = np.lib.stride_tricks.sliding_window_view(kp, (KS, KS, KS),
                                                   axis=(0, 1, 2))
    # win: [40,40,40,48,3,3,3] -> [40,40,40,(kk,c)]
    win = np.ascontiguousarray(win.transpose(0, 1, 2, 4, 5, 6, 3))
    win = win.reshape(H, W, T, FKC)

    rpb_kh = np.ascontiguousarray(rpb.reshape(NH, NT).T).reshape(FKH)
    rpb_t = np.broadcast_to(rpb_kh, (P, FKH)).copy()

    in_maps = []
    for i in range(N_CORES):
        h0 = i * SLAB
        q_pad = np.zeros((TOKP, DIM), np.float32)
        q_pad[:TOK] = q0[h0:h0 + SLAB].reshape(TOK, DIM)
        kn_pad = np.zeros((TOKP, FKC), np.float32)
        kn_pad[:TOK] = win[h0:h0 + SLAB].reshape(TOK, FKC)
        in_maps.append({
            "qs": q_pad.reshape(TILES, P, TPP * DIM),
            "kn": kn_pad.reshape(TILES, P, TPP * FKC),
            "rpbt": rpb_t,
        })
    return in_maps


def _assemble(results):
    slabs = []
    for i in range(N_CORES):
        o = results[i]["out"].reshape(TILES, P, 3, TPP, NH)
        o = o.transpose(0, 1, 3, 2, 4).reshape(TOKP, 3, NH)[:TOK]
        o = o.reshape(SLAB, W, T, 3, NH)
        # channel order in reference: c = h*3 + o
        slabs.append(o.transpose(0, 1, 2, 4, 3).reshape(SLAB, W, T, 3 * NH))
    full = np.concatenate(slabs, axis=0)             # [40,40,40,24]
    return np.ascontiguousarray(full.transpose(3, 0, 1, 2))[None]


def _run(q, k, rpb, **spmd_kwargs):
    if "prog" not in _prog_cache:
        _prog_cache["prog"] = _build_program()
    nc = _prog_cache["prog"]
    in_maps = _host_prep(q, k, rpb)
    res = run_bass_kernel_spmd(nc, in_maps, list(range(N_CORES)),
                               **spmd_kwargs)
    return _assemble(res.results), res


def kernel(q, k, rpb):
    out, _ = _run(q, k, rpb)
    return out



# revision 5
# speedup vs baseline: 2.3287x; 2.3287x over previous
"""Trainium2 Bass kernel for 3D neighborhood attention (sparse_attention).

Problem: q,k [1,40,40,40,48] fp32, rpb [8,3,3,3]; out [1,24,40,40,40].
Per voxel x: logits[h,kk] = scale * <q[x,h,:], k[x+off_kk,h,:]> + rpb[h,kk]
(zero-padded k at boundaries, kk over 3x3x3 offsets), p = softmax over kk,
out[x,h,:] = sum_kk p[h,kk] * off_kk  (constant integer offsets as values).

Sharding: spatial-parallel over H (40 -> 8 slabs of 5). Each core gets its
q slab plus a host-side im2col of the 27 shifted k views for its slab
(halo handled on host): on-core everything is token-parallel with tokens
on SBUF partitions (8 tokens per partition, 8 tiles of 1024 tokens).

v2 layout: fp16 end-to-end on the hot path so every DVE tensor_tensor op
runs in 2x_1P packed mode, with the head-dim (d) OUTERMOST in the free
axis so the QK d-reduction is three contiguous fold-adds (2x) instead of
a 1x tensor_reduce. The softmax phase computes the denominator and the
three directional numerators via a factorized (di -> dj -> dl) fold tree:
the "values" are the constant offsets in {-1,0,1}^3, so out_i =
(sum over +1 slab) - (sum over -1 slab), and all four contractions share
partial sums.
"""

import numpy as np

import concourse.bass as bass
import concourse.tile as tile
from concourse import bacc, mybir
from concourse.bass_utils import run_bass_kernel_spmd

F16 = np.float16

NH = 8
HD = 6
DIM = NH * HD
KS = 3
NT = KS**3  # 27
SCALE = HD**-0.5
H = W = T = 40
N_CORES = 8
SLAB = H // N_CORES          # 5 rows of H per core
TOK = SLAB * W * T           # 8000 tokens per core
P = 128
TPP = 8                      # tokens per partition per tile
TILES = 8                    # 8 * 128 * 8 = 8192 >= 8000
TOKP = TILES * P * TPP       # 8192
FKC = NT * DIM               # 1296 products per token
FKH = NT * NH                # 216 logits per token

_prog_cache = {}


def _build_program():
    fp16 = mybir.dt.float16
    fp32 = mybir.dt.float32
    nc = bacc.Bacc("TRN2", target_bir_lowering=False, debug=False,
                   num_devices=N_CORES)
    # free layouts (per partition):
    #   qs : (d6, j8, h8)             = 384
    #   kn : (d6, j8, kk27, h8)       = 10368
    #   rpb: (kk27, h8)               = 216
    #   out: (o3, j8, h8)             = 192   (o = di,dj,dl numerators / s0)
    qs = nc.dram_tensor("qs", [TILES, P, HD * TPP * NH], fp16,
                        kind="ExternalInput").ap()
    kn = nc.dram_tensor("kn", [TILES, P, HD * TPP * NT * NH], fp16,
                        kind="ExternalInput").ap()
    rpbt = nc.dram_tensor("rpbt", [P, FKH], fp16, kind="ExternalInput").ap()
    out = nc.dram_tensor("out", [TILES, P, 3 * TPP * NH], fp32,
                         kind="ExternalOutput").ap()

    J = TPP
    NJH = TPP * NT * NH          # 1728 logits per partition
    D3 = 3 * TPP * NT * NH      # 5184: one 3-fold of products

    with tile.TileContext(nc) as tc:
        with (
            tc.tile_pool(name="consts", bufs=1) as cpool,
            tc.tile_pool(name="kin", bufs=2) as kpool,
            tc.tile_pool(name="qin", bufs=2) as qpool,
            tc.tile_pool(name="prod", bufs=1) as ppool,
            tc.tile_pool(name="fold", bufs=1) as fpool,
            tc.tile_pool(name="f2at", bufs=2) as f2pool,
            tc.tile_pool(name="lt", bufs=2) as ltpool,
            tc.tile_pool(name="expv", bufs=2) as epool,
            tc.tile_pool(name="l1", bufs=2) as l1pool,
            tc.tile_pool(name="l3", bufs=2) as l3pool,
            tc.tile_pool(name="tt", bufs=2) as ttpool,
            tc.tile_pool(name="small", bufs=8) as spool,
            tc.tile_pool(name="outp", bufs=2) as opool,
        ):
            rpb_sb = cpool.tile([P, FKH], fp16)
            nc.sync.dma_start(rpb_sb[:], rpbt[:])

            for ti in range(TILES):
                kt = kpool.tile([P, HD * TPP * NT * NH], fp16)
                nc.sync.dma_start(kt[:], kn[ti])
                qt = qpool.tile([P, HD * TPP * NH], fp16)
                nc.sync.dma_start(qt[:], qs[ti])

                # P4[p, d, j, kk, h] = kn * q (q broadcast over kk)
                pt = ppool.tile([P, HD * TPP * NT * NH], fp16)
                q_b = (qt[:].rearrange("p (d j h) -> p d j h", d=HD, j=J)
                       .unsqueeze(3).broadcast_to([P, HD, J, NT, NH]))
                nc.vector.tensor_mul(
                    pt[:].rearrange("p (d j kk h) -> p d j kk h", d=HD, j=J,
                                    kk=NT),
                    kt[:].rearrange("p (d j kk h) -> p d j kk h", d=HD, j=J,
                                    kk=NT),
                    q_b,
                )
                # d-reduction by contiguous folds: 6 -> 3 -> (2+1)
                f1 = fpool.tile([P, D3], fp16)
                nc.vector.tensor_add(f1[:], pt[:, :D3], pt[:, D3:])
                f2 = f2pool.tile([P, NJH], fp16)
                nc.vector.tensor_add(f2[:], f1[:, :NJH], f1[:, NJH:2 * NJH])
                # at = third d-fold + rpb (broadcast over j)
                at = f2pool.tile([P, NJH], fp16)
                rpb_b = rpb_sb[:].unsqueeze(1).broadcast_to([P, J, FKH])
                nc.vector.tensor_add(
                    at[:].rearrange("p (j f) -> p j f", j=J),
                    f1[:, 2 * NJH:].rearrange("p (j f) -> p j f", j=J),
                    rpb_b,
                )
                lt = ltpool.tile([P, NJH], fp16)
                nc.vector.tensor_add(lt[:], f2[:], at[:])

                # E = exp(L) on ScalarE (overlaps with DVE)
                et = epool.tile([P, NJH], fp16)
                nc.scalar.activation(et[:], lt[:],
                                     mybir.ActivationFunctionType.Exp)

                # Factorized contractions over kk = (di, dj, dl):
                # level 1 (contract di): a0 = sum_di E, a1 = E[di2]-E[di0]
                ev = et[:].rearrange("p (j di r) -> p j di r", j=J, di=KS)
                tt = ttpool.tile([P, J * 72], fp16)
                tv = tt[:].rearrange("p (j r) -> p j r", j=J)
                nc.vector.tensor_add(tv, ev[:, :, 0], ev[:, :, 1])
                l1t = l1pool.tile([P, 2 * J * 72], fp16)  # (s2, j, dj, dl, h)
                a0f = l1t[:, :J * 72].rearrange("p (j r) -> p j r", j=J)
                a1f = l1t[:, J * 72:].rearrange("p (j r) -> p j r", j=J)
                nc.vector.tensor_add(a0f, tv, ev[:, :, 2])
                nc.vector.tensor_sub(a1f, ev[:, :, 2], ev[:, :, 0])

                # level 2 (contract dj) for a0 and a1 together
                lv = l1t[:].rearrange("p (s j dj r) -> p s j dj r", s=2, j=J,
                                      dj=KS)
                ut = spool.tile([P, 2 * J * 24], fp16)
                uv = ut[:].rearrange("p (s j r) -> p s j r", s=2, j=J)
                nc.vector.tensor_add(uv, lv[:, :, :, 0], lv[:, :, :, 1])
                # l3in slots: s=0: B0=sum_dj a0, s=1: C1=sum_dj a1, s=2: B1
                l3in = l3pool.tile([P, 3 * J * 24], fp16)
                sall = l3in[:, :2 * J * 24].rearrange("p (s j r) -> p s j r",
                                                      s=2, j=J)
                nc.vector.tensor_add(sall, uv, lv[:, :, :, 2])
                a0v = l1t[:, :J * 72].rearrange("p (j dj r) -> p j dj r",
                                                j=J, dj=KS)
                b1f = l3in[:, 2 * J * 24:].rearrange("p (j r) -> p j r", j=J)
                nc.vector.tensor_sub(b1f, a0v[:, :, 2], a0v[:, :, 0])

                # level 3 (contract dl): zt slots = (s0, N_di, N_dj, N_dl)
                l3v = l3in[:].rearrange("p (s j dl h) -> p s j dl h", s=3,
                                        j=J, dl=KS)
                wt = spool.tile([P, 3 * J * NH], fp16)
                wv = wt[:].rearrange("p (s j h) -> p s j h", s=3, j=J)
                nc.vector.tensor_add(wv, l3v[:, :, :, 0], l3v[:, :, :, 1])
                zt = spool.tile([P, 4 * J * NH], fp16)
                zv = zt[:, :3 * J * NH].rearrange("p (s j h) -> p s j h",
                                                  s=3, j=J)
                nc.vector.tensor_add(zv, wv, l3v[:, :, :, 2])
                b0v = l3v[:, 0]  # [p, j, dl, h]
                ndl = zt[:, 3 * J * NH:].rearrange("p (j h) -> p j h", j=J)
                nc.vector.tensor_sub(ndl, b0v[:, :, 2], b0v[:, :, 0])

                # out[o, j, h] = N_o / s0
                rt = spool.tile([P, J * NH], fp32)
                nc.vector.reciprocal(rt[:], zt[:, :J * NH])
                ot = opool.tile([P, 3 * TPP * NH], fp32)
                r_b = (rt[:].rearrange("p (j h) -> p j h", j=J)
                       .unsqueeze(1).broadcast_to([P, 3, J, NH]))
                nc.vector.tensor_mul(
                    ot[:].rearrange("p (o j h) -> p o j h", o=3, j=J),
                    zt[:, J * NH:].rearrange("p (o j h) -> p o j h", o=3,
                                             j=J),
                    r_b,
                )
                nc.sync.dma_start(out[ti], ot[:])

    nc.compile()
    return nc


def _host_prep(q, k, rpb):
    q = np.asarray(q, dtype=np.float32)
    k = np.asarray(k, dtype=np.float32)
    rpb = np.asarray(rpb, dtype=np.float32)

    q0 = (q[0] * SCALE).astype(F16)                 # [40,40,40,48]
    kp = np.pad(k[0], ((1, 1), (1, 1), (1, 1), (0, 0)))  # [42,42,42,48]
    win = np.lib.stride_tricks.sliding_window_view(kp, (KS, KS, KS),
                                                   axis=(0, 1, 2))
    # win: [40,40,40,48,3,3,3] -> [40,40,40,kk,48]
    win = win.transpose(0, 1, 2, 4, 5, 6, 3).reshape(H, W, T, NT, DIM)

    rpb_kh = np.ascontiguousarray(rpb.reshape(NH, NT).T).reshape(FKH)
    rpb_t = np.broadcast_to(rpb_kh.astype(F16), (P, FKH)).copy()

    in_maps = []
    for i in range(N_CORES):
        h0 = i * SLAB
        # tokens -> (tile, partition, j); free layouts are d-major
        q_pad = np.zeros((TOKP, NH, HD), F16)
        q_pad[:TOK] = q0[h0:h0 + SLAB].reshape(TOK, NH, HD)
        # [ti, p, j, h, d] -> [ti, p, d, j, h]
        q_t = np.ascontiguousarray(
            q_pad.reshape(TILES, P, TPP, NH, HD).transpose(0, 1, 4, 2, 3)
        ).reshape(TILES, P, HD * TPP * NH)

        kn_pad = np.zeros((TOKP, NT, NH, HD), F16)
        kn_pad[:TOK] = win[h0:h0 + SLAB].reshape(TOK, NT, NH, HD)
        # [ti, p, j, kk, h, d] -> [ti, p, d, j, kk, h]
        kn_t = np.ascontiguousarray(
            kn_pad.reshape(TILES, P, TPP, NT, NH, HD)
            .transpose(0, 1, 5, 2, 3, 4)
        ).reshape(TILES, P, HD * TPP * NT * NH)

        in_maps.append({"qs": q_t, "kn": kn_t, "rpbt": rpb_t})
    return in_maps


def _assemble(results):
    slabs = []
    for i in range(N_CORES):
        o = results[i]["out"].reshape(TILES, P, 3, TPP, NH)
        o = o.transpose(0, 1, 3, 2, 4).reshape(TOKP, 3, NH)[:TOK]
        o = o.reshape(SLAB, W, T, 3, NH)
        # channel order in reference: c = h*3 + o
        slabs.append(o.transpose(0, 1, 2, 4, 3).reshape(SLAB, W, T, 3 * NH))
    full = np.concatenate(slabs, axis=0)             # [40,40,40,24]
    return np.ascontiguousarray(full.transpose(3, 0, 1, 2))[None]


def _run(q, k, rpb, **spmd_kwargs):
    if "prog" not in _prog_cache:
        _prog_cache["prog"] = _build_program()
    nc = _prog_cache["prog"]
    in_maps = _host_prep(q, k, rpb)
    res = run_bass_kernel_spmd(nc, in_maps, list(range(N_CORES)),
                               **spmd_kwargs)
    return _assemble(res.results), res


def kernel(q, k, rpb):
    out, _ = _run(q, k, rpb)
    return out
